# revision 25
# baseline (speedup 1.0000x reference)
import sys

if "/opt/trn_rl_repo" not in sys.path:
    sys.path.insert(0, "/opt/trn_rl_repo")

import numpy as np

B, T, C = 2, 2048, 2048
H, H_KV = 16, 8
D = C // H  # 128
NCORES = 8
HL = H // NCORES  # 2 local query heads per core; 1 kv head per core

F32R_SCALE = 0.08838834764831845  # 1/sqrt(128)


def build_nc(b=B, t=T, c=C, mmdt="f32r"):
    """Build the per-core Bass program. Same program on all 8 cores; the
    sharding lives entirely in the input data each core receives."""
    import concourse.bass as bass  # noqa: F401
    import concourse.mybir as mybir
    import concourse.tile as tile
    from concourse import bacc

    f32 = mybir.dt.float32
    f32r = mybir.dt.float32r if mmdt == "f32r" else mybir.dt.bfloat16
    EXP = mybir.ActivationFunctionType.Exp

    ncb = c // 128  # contraction blocks for projections
    nt = t // 512  # 512-wide t tiles
    njb_per_t = 512 // 128  # 4 k-blocks per 512 q-tile

    nc = bacc.Bacc("TRN2", target_bir_lowering=False, debug=False)

    xT = nc.dram_tensor("xT", [b, c, t], f32r, kind="ExternalInput")
    wq = nc.dram_tensor("wq", [c, HL * D], f32r, kind="ExternalInput")
    wk = nc.dram_tensor("wk", [c, D], f32r, kind="ExternalInput")
    wv = nc.dram_tensor("wv", [c, D], f32r, kind="ExternalInput")
    wp = nc.dram_tensor("wp", [HL * D, c], f32r, kind="ExternalInput")
    cos2 = nc.dram_tensor("cos2", [128, t], f32, kind="ExternalInput")
    sin2 = nc.dram_tensor("sin2", [128, t], f32, kind="ExternalInput")
    maskf = nc.dram_tensor("maskf", [128, 512], f32r, kind="ExternalInput")
    onesv = nc.dram_tensor("onesv", [128, 1], f32r, kind="ExternalInput")
    ident = nc.dram_tensor("ident", [128, 128], f32, kind="ExternalInput")
    y = nc.dram_tensor("y", [b, t, c], f32, kind="ExternalOutput")

    with tile.TileContext(nc) as tc:
        with (
            tc.tile_pool(name="wts", bufs=1) as wpool,
            tc.tile_pool(name="data", bufs=1) as dpool,
            tc.tile_pool(name="work", bufs=2) as wkp,
            tc.tile_pool(name="psum", bufs=1, space="PSUM") as pp,
        ):
            # ---- resident weights / tables ----
            # prefetch the first t-tile's leading xt chunks before the
            # weight bulk so the first projection matmuls start ASAP
            xt_pre = {}
            for cb in range(min(6, ncb)):
                xtp = wkp.tile([128, 512], f32r, tag="xt", bufs=8, name=f"xtp{cb}")
                nc.sync.dma_start(xtp[:], xT[0, cb * 128 : (cb + 1) * 128, 0:512])
                xt_pre[(0, 0, cb)] = xtp

            nw = max(ncb // 4, 1)  # cb chunks per weight DMA
            wq_sbs, wk_sbs, wv_sbs = [], [], []
            for wi in range(ncb // nw):
                cbs = slice(wi * nw * 128, (wi + 1) * nw * 128)
                wq_i = wpool.tile([128, nw * HL * D], f32r, name=f"wq{wi}")
                nc.sync.dma_start(
                    wq_i[:].rearrange("p (cb d) -> p cb d", d=HL * D),
                    wq[cbs, :].rearrange("(cb p) d -> p cb d", p=128),
                )
                wq_sbs.append(wq_i)
                wk_i = wpool.tile([128, nw * D], f32r, name=f"wk{wi}")
                nc.sync.dma_start(
                    wk_i[:].rearrange("p (cb d) -> p cb d", d=D),
                    wk[cbs, :].rearrange("(cb p) d -> p cb d", p=128),
                )
                wk_sbs.append(wk_i)
                wv_i = wpool.tile([128, nw * D], f32r, name=f"wv{wi}")
                nc.sync.dma_start(
                    wv_i[:].rearrange("p (cb d) -> p cb d", d=D),
                    wv[cbs, :].rearrange("(cb p) d -> p cb d", p=128),
                )
                wv_sbs.append(wv_i)
            wp_sb = wpool.tile([128, HL * c], f32r)  # [p, (f, cout)]
            nc.scalar.dma_start(
                wp_sb[:].rearrange("p (f n) -> p f n", n=c),
                wp.rearrange("(f p) n -> p f n", p=128),
            )
            cos_sb = wpool.tile([128, t], f32)
            nc.scalar.dma_start(cos_sb[:], cos2[:, :])
            sin_sb = wpool.tile([128, t], f32)
            nc.scalar.dma_start(sin_sb[:], sin2[:, :])
            mask_sb = wpool.tile([128, 512], f32r)
            nc.scalar.dma_start(mask_sb[:], maskf[:, :])
            ones_sb = wpool.tile([128, 1], f32r)
            nc.scalar.dma_start(ones_sb[:], onesv[:, :])
            id_sb = wpool.tile([128, 128], f32)
            nc.scalar.dma_start(id_sb[:], ident[:, :])

            for bi in range(b):
                # ---- per-batch persistent tiles ----
                QT = [dpool.tile([128, t], f32r, tag=f"qt{h}", name=f"QT{h}") for h in range(HL)]
                KT = dpool.tile([128, t], f32r, tag="kt")
                VT = dpool.tile([128, t], f32, tag="vtt")
                Vn = dpool.tile([128, t], f32r, tag="vn")  # V natural [k, (jb d)]
                AT = [dpool.tile([128, t], f32r, tag=f"at{h}", name=f"AT{h}") for h in range(HL)]

                # ---- QKV projections (+ fused RoPE for Q, K) ----
                for i4 in range(nt):
                    ts_ = slice(i4 * 512, (i4 + 1) * 512)
                    ps = {
                        kind: pp.tile([128, 512], f32, tag="proj", bufs=4, name=f"ps_{kind}")
                        for kind in ("q0", "q1", "k", "v")
                    }
                    for cb in range(ncb):
                        if (bi, i4, cb) in xt_pre:
                            xt = xt_pre.pop((bi, i4, cb))
                        else:
                            xt = wkp.tile([128, 512], f32r, tag="xt", bufs=8)
                            nc.sync.dma_start(xt[:], xT[bi, cb * 128 : (cb + 1) * 128, ts_])
                        xtr = xt[:]
                        st, sp = (cb == 0), (cb == ncb - 1)
                        wi, cbl = cb // nw, cb % nw
                        base = cbl * HL * D
                        nc.tensor.matmul(
                            ps["q0"][:],
                            wq_sbs[wi][:, base : base + 128],
                            xtr, start=st, stop=sp,
                        )
                        nc.tensor.matmul(
                            ps["q1"][:],
                            wq_sbs[wi][:, base + 128 : base + 256],
                            xtr, start=st, stop=sp,
                        )
                        nc.tensor.matmul(
                            ps["k"][:],
                            wk_sbs[wi][:, cbl * 128 : (cbl + 1) * 128],
                            xtr, start=st, stop=sp,
                        )
                        nc.tensor.matmul(
                            ps["v"][:],
                            wv_sbs[wi][:, cbl * 128 : (cbl + 1) * 128],
                            xtr, start=st, stop=sp,
                        )
                    # RoPE in pair-interleaved head layout (host permuted Wq/Wk
                    # columns so rotate-half pairs are adjacent partitions):
                    # dest = psum*cosI + swap_adjacent(psum)*sinS
                    swap_mask = [i ^ 1 for i in range(32)]
                    for kind, dest in (("q0", QT[0]), ("q1", QT[1]), ("k", KT)):
                        ra = wkp.tile([128, 512], f32, tag="ra", bufs=2)
                        rb = wkp.tile([128, 512], f32, tag="rb", bufs=2)
                        nc.vector.tensor_mul(ra[:], ps[kind][:], cos_sb[:, ts_])
                        nc.vector.stream_shuffle(rb[:], ps[kind][:], swap_mask)
                        nc.vector.tensor_mul(rb[:], rb[:], sin_sb[:, ts_])
                        nc.vector.tensor_add(dest[:, ts_], ra[:], rb[:])
                    nc.vector.tensor_copy(VT[:, ts_], ps["v"][:])

                # ---- V natural ([t,d] blocks) via PE transpose ----
                for j in range(t // 128):
                    pt = pp.tile([128, 128], f32, tag="av", bufs=1)
                    nc.tensor.transpose(pt[:], VT[:, j * 128 : (j + 1) * 128], id_sb[:])
                    nc.vector.tensor_copy(Vn[:, j * 128 : (j + 1) * 128], pt[:])

                # ---- attention (+ interleaved output projection) ----
                for i4 in range(nt):
                    for h in range(HL):
                        qs = slice(i4 * 512, (i4 + 1) * 512)
                        pav = pp.tile([128, 512], f32, tag="av", bufs=1)
                        pden = pp.tile([1, 512], f32, tag="den", bufs=1)
                        jmax = njb_per_t * (i4 + 1) - 1
                        for j in range(jmax + 1):
                            diag = j - njb_per_t * i4
                            off = max(diag, 0) * 128  # skip q cols left of diag
                            pst = pp.tile([128, 512], f32, tag="s", bufs=2)
                            nc.tensor.matmul(
                                pst[:, off:512],
                                KT[:, j * 128 : (j + 1) * 128],
                                QT[h][:, i4 * 512 + off : (i4 + 1) * 512],
                                start=True, stop=True,
                            )
                            E = wkp.tile([128, 512], f32r, tag="E", bufs=4)
                            nc.scalar.activation(
                                E[:, off:512], pst[:, off:512], EXP, scale=F32R_SCALE
                            )
                            if diag >= 0:
                                # zero strictly-lower triangle of the diag block
                                nc.vector.tensor_mul(
                                    E[:, off : off + 128],
                                    E[:, off : off + 128],
                                    mask_sb[:, 384:512],
                                )
                            nc.tensor.matmul(
                                pav[:, off:512],
                                Vn[:, j * 128 : (j + 1) * 128],
                                E[:, off:512],
                                start=(j == 0), stop=(j == jmax),
                                skip_group_check=True,
                            )
                            nc.tensor.matmul(
                                pden[:, off:512],
                                ones_sb[:, 0:1],
                                E[:, off:512],
                                start=(j == 0), stop=(j == jmax),
                                skip_group_check=True,
                            )
                        rec = wkp.tile([1, 512], f32, tag="rec", bufs=2)
                        nc.vector.reciprocal_approx_fast(rec[:], pden[:])
                        rbc = wkp.tile([128, 512], f32, tag="rbc", bufs=2)
                        nc.gpsimd.partition_broadcast(rbc[:], rec[:])
                        nc.vector.tensor_mul(AT[h][:, qs], pav[:], rbc[:])

                    # ---- output projection for the rows this i4 finished ----
                    for it in range(i4 * 4, (i4 + 1) * 4):
                        for n in range(c // 512):
                            po = pp.tile([128, 512], f32, tag="proj", bufs=4)
                            for hh in range(HL):
                                nc.tensor.matmul(
                                    po[:],
                                    AT[hh][:, it * 128 : (it + 1) * 128],
                                    wp_sb[:, hh * c + n * 512 : hh * c + (n + 1) * 512],
                                    start=(hh == 0), stop=(hh == HL - 1),
                                )
                            po_sb = wkp.tile([128, 512], f32, tag="yout", bufs=4)
                            if (it * (c // 512) + n) % 2 == 0:
                                nc.vector.tensor_copy(po_sb[:], po[:])
                            else:
                                nc.scalar.copy(po_sb[:], po[:])
                            nc.sync.dma_start(
                                y[bi, it * 128 : (it + 1) * 128, n * 512 : (n + 1) * 512],
                                po_sb[:],
                            )

    nc.compile()
    return nc


def host_inputs(x, Wq, Wk, Wv, Wp, ncores=NCORES, mmdt="f32r"):
    import ml_dtypes

    mdt = np.float32 if mmdt == "f32r" else ml_dtypes.bfloat16
    """Per-core input dicts (sharding + layout prep on host)."""
    b, t, c = x.shape
    d = D
    xT = np.ascontiguousarray(np.transpose(x, (0, 2, 1)))  # [B, C, T]
    inv = (1.0 / (10000.0 ** (np.arange(0, d, 2, dtype=np.float32) / np.float32(d)))).astype(np.float32)
    pos = np.arange(t, dtype=np.float32)
    fr = np.outer(pos, inv).astype(np.float32)  # [T, 64]
    cosT = np.cos(fr).T.astype(np.float32)  # [64, T]
    sinT = np.sin(fr).T.astype(np.float32)
    # pair-interleaved rope tables: partition 2m,2m+1 <- freq m; sign -/+ on sin
    cosI = np.ascontiguousarray(np.repeat(cosT, 2, axis=0))  # [128, T]
    sinS = np.ascontiguousarray(
        np.stack([-sinT, sinT], axis=1).reshape(128, t)
    )
    # column permutation putting rope pair (m, m+64) at (2m, 2m+1), per head
    perm = np.stack([np.arange(64), np.arange(64) + 64], 1).reshape(128)
    triu = np.triu(np.ones((128, 128), np.float32))
    maskf = np.ascontiguousarray(
        np.concatenate([np.zeros((128, 384), np.float32), triu], 1)
    )
    onesv = np.ones((128, 1), np.float32)
    ident = np.eye(128, dtype=np.float32)

    def permute_heads(w):
        # w: [c, nheads*d] -> same with each head's columns permuted by perm
        nh = w.shape[1] // d
        wv_ = w.reshape(w.shape[0], nh, d)
        return np.ascontiguousarray(wv_[:, :, perm].reshape(w.shape))

    Wq_p = permute_heads(Wq)
    Wk_p = permute_heads(Wk)

    xTm = xT.astype(mdt) if mdt is not np.float32 else xT
    in_maps = []
    for ci in range(ncores):
        qs = slice(ci * HL * d, (ci + 1) * HL * d)
        in_maps.append(
            {
                "xT": xTm,
                "wq": np.ascontiguousarray(Wq_p[:, qs]).astype(mdt),
                "wk": np.ascontiguousarray(Wk_p[:, ci * d : (ci + 1) * d]).astype(mdt),
                "wv": np.ascontiguousarray(Wv[:, ci * d : (ci + 1) * d]).astype(mdt),
                "wp": np.ascontiguousarray(Wp[qs, :]).astype(mdt),
                "cos2": cosI,
                "sin2": sinS,
                "maskf": maskf.astype(mdt),
                "onesv": onesv.astype(mdt),
                "ident": ident,
            }
        )
    return in_maps


_NC_CACHE = {}

MMDT = "f32r"


def _get_nc(mmdt=None):
    mmdt = mmdt or MMDT
    key = (B, T, C, mmdt)
    if key not in _NC_CACHE:
        _NC_CACHE[key] = build_nc(B, T, C, mmdt=mmdt)
    return _NC_CACHE[key]


def _install_cc_error_surfacing():
    """Make neuronx_cc hook failures print a real traceback instead of the
    opaque PJRT 'py_result' error."""
    try:
        from concourse import bass2jax

        bass2jax.install_neuronx_cc_hook()
        import libneuronxla

        if getattr(libneuronxla, "_tb_wrapped", False):
            return
        inner = libneuronxla.neuronx_cc

        def wrapped(*a, **k):
            try:
                return inner(*a, **k)
            except BaseException:
                import traceback

                traceback.print_exc()
                raise

        libneuronxla.neuronx_cc = wrapped
        libneuronxla._tb_wrapped = True
    except Exception:
        pass


def run_spmd(x, Wq, Wk, Wv, Wp, trace=False, mmdt=None):
    from concourse.bass_utils import run_bass_kernel_spmd

    mmdt = mmdt or MMDT
    _install_cc_error_surfacing()

    nc = _get_nc(mmdt)
    in_maps = host_inputs(x, Wq, Wk, Wv, Wp, mmdt=mmdt)
    last_err = None
    for attempt in range(3):
        try:
            res = run_bass_kernel_spmd(
                nc, in_maps, core_ids=list(range(NCORES)), trace=trace
            )
            break
        except Exception as e:  # transient NRT device faults: retry
            last_err = e
            import time as _time

            _time.sleep(5.0)
    else:
        raise last_err
    acc = res.results[0]["y"].astype(np.float64)
    for i in range(1, NCORES):
        acc += res.results[i]["y"]
    return acc.astype(np.float32), res


def kernel(x, Wq, Wk, Wv, Wp):
    out, _ = run_spmd(x, Wq, Wk, Wv, Wp, trace=False)
    return out


# revision 26
# speedup vs baseline: 1.0237x; 1.0237x over previous
import sys

if "/opt/trn_rl_repo" not in sys.path:
    sys.path.insert(0, "/opt/trn_rl_repo")

import numpy as np

B, T, C = 2, 2048, 2048
H, H_KV = 16, 8
D = C // H  # 128
NCORES = 8
HL = H // NCORES  # 2 local query heads per core; 1 kv head per core

F32R_SCALE = 0.08838834764831845  # 1/sqrt(128)


def build_nc(b=B, t=T, c=C, mmdt="f32r"):
    """Build the per-core Bass program. Same program on all 8 cores; the
    sharding lives entirely in the input data each core receives."""
    import concourse.bass as bass  # noqa: F401
    import concourse.mybir as mybir
    import concourse.tile as tile
    from concourse import bacc

    f32 = mybir.dt.float32
    f32r = mybir.dt.float32r if mmdt == "f32r" else mybir.dt.bfloat16
    EXP = mybir.ActivationFunctionType.Exp

    ncb = c // 128  # contraction blocks for projections
    nt = t // 512  # 512-wide t tiles
    njb_per_t = 512 // 128  # 4 k-blocks per 512 q-tile

    nc = bacc.Bacc("TRN2", target_bir_lowering=False, debug=False)

    xT = nc.dram_tensor("xT", [b, c, t], f32r, kind="ExternalInput")
    wq = nc.dram_tensor("wq", [c, HL * D], f32r, kind="ExternalInput")
    wk = nc.dram_tensor("wk", [c, D], f32r, kind="ExternalInput")
    wv = nc.dram_tensor("wv", [c, D], f32r, kind="ExternalInput")
    wp = nc.dram_tensor("wp", [HL * D, c], f32r, kind="ExternalInput")
    cos2 = nc.dram_tensor("cos2", [128, t], f32, kind="ExternalInput")
    sin2 = nc.dram_tensor("sin2", [128, t], f32, kind="ExternalInput")
    maskf = nc.dram_tensor("maskf", [128, 512], f32r, kind="ExternalInput")
    onesv = nc.dram_tensor("onesv", [128, 1], f32r, kind="ExternalInput")
    ident = nc.dram_tensor("ident", [128, 128], f32, kind="ExternalInput")
    y = nc.dram_tensor("y", [b, t, c], f32, kind="ExternalOutput")

    with tile.TileContext(nc) as tc:
        with (
            tc.tile_pool(name="wts", bufs=1) as wpool,
            tc.tile_pool(name="data", bufs=1) as dpool,
            tc.tile_pool(name="work", bufs=2) as wkp,
            tc.tile_pool(name="psum", bufs=1, space="PSUM") as pp,
        ):
            # ---- resident weights / tables ----
            # prefetch the first t-tile's leading xt chunks before the
            # weight bulk so the first projection matmuls start ASAP
            xt_pre = {}
            for cb in range(min(6, ncb)):
                xtp = wkp.tile([128, 512], f32r, tag="xt", bufs=8, name=f"xtp{cb}")
                nc.sync.dma_start(xtp[:], xT[0, cb * 128 : (cb + 1) * 128, 0:512])
                xt_pre[(0, 0, cb)] = xtp

            nw = max(ncb // 4, 1)  # cb chunks per weight DMA
            wq_sbs, wk_sbs, wv_sbs = [], [], []
            for wi in range(ncb // nw):
                cbs = slice(wi * nw * 128, (wi + 1) * nw * 128)
                wq_i = wpool.tile([128, nw * HL * D], f32r, name=f"wq{wi}")
                nc.sync.dma_start(
                    wq_i[:].rearrange("p (cb d) -> p cb d", d=HL * D),
                    wq[cbs, :].rearrange("(cb p) d -> p cb d", p=128),
                )
                wq_sbs.append(wq_i)
                wk_i = wpool.tile([128, nw * D], f32r, name=f"wk{wi}")
                nc.sync.dma_start(
                    wk_i[:].rearrange("p (cb d) -> p cb d", d=D),
                    wk[cbs, :].rearrange("(cb p) d -> p cb d", p=128),
                )
                wk_sbs.append(wk_i)
                wv_i = wpool.tile([128, nw * D], f32r, name=f"wv{wi}")
                nc.sync.dma_start(
                    wv_i[:].rearrange("p (cb d) -> p cb d", d=D),
                    wv[cbs, :].rearrange("(cb p) d -> p cb d", p=128),
                )
                wv_sbs.append(wv_i)
            wp_sb = wpool.tile([128, HL * c], f32r)  # [p, (f, cout)]
            nc.scalar.dma_start(
                wp_sb[:].rearrange("p (f n) -> p f n", n=c),
                wp.rearrange("(f p) n -> p f n", p=128),
            )
            cos_sb = wpool.tile([128, t], f32)
            nc.scalar.dma_start(cos_sb[:], cos2[:, :])
            sin_sb = wpool.tile([128, t], f32)
            nc.scalar.dma_start(sin_sb[:], sin2[:, :])
            mask_sb = wpool.tile([128, 512], f32r)
            nc.scalar.dma_start(mask_sb[:], maskf[:, :])
            ones_sb = wpool.tile([128, 1], f32r)
            nc.scalar.dma_start(ones_sb[:], onesv[:, :])
            id_sb = wpool.tile([128, 128], f32)
            nc.scalar.dma_start(id_sb[:], ident[:, :])

            for bi in range(b):
                # ---- per-batch persistent tiles ----
                QT = [dpool.tile([128, t], f32r, tag=f"qt{h}", name=f"QT{h}") for h in range(HL)]
                KT = dpool.tile([128, t], f32r, tag="kt")
                VT = dpool.tile([128, t], f32, tag="vtt")
                Vn = dpool.tile([128, t], f32r, tag="vn")  # V natural [k, (jb d)]
                AT = [dpool.tile([128, t], f32r, tag=f"at{h}", name=f"AT{h}") for h in range(HL)]

                # ---- QKV projections (+ fused RoPE for Q, K) ----
                for i4 in range(nt):
                    ts_ = slice(i4 * 512, (i4 + 1) * 512)
                    ps = {
                        kind: pp.tile([128, 512], f32, tag="proj", bufs=4, name=f"ps_{kind}")
                        for kind in ("q0", "q1", "k", "v")
                    }
                    for cb in range(ncb):
                        if (bi, i4, cb) in xt_pre:
                            xt = xt_pre.pop((bi, i4, cb))
                        else:
                            xt = wkp.tile([128, 512], f32r, tag="xt", bufs=8)
                            nc.sync.dma_start(xt[:], xT[bi, cb * 128 : (cb + 1) * 128, ts_])
                        xtr = xt[:]
                        st, sp = (cb == 0), (cb == ncb - 1)
                        wi, cbl = cb // nw, cb % nw
                        base = cbl * HL * D
                        nc.tensor.matmul(
                            ps["q0"][:],
                            wq_sbs[wi][:, base : base + 128],
                            xtr, start=st, stop=sp,
                        )
                        nc.tensor.matmul(
                            ps["q1"][:],
                            wq_sbs[wi][:, base + 128 : base + 256],
                            xtr, start=st, stop=sp,
                        )
                        nc.tensor.matmul(
                            ps["k"][:],
                            wk_sbs[wi][:, cbl * 128 : (cbl + 1) * 128],
                            xtr, start=st, stop=sp,
                        )
                        nc.tensor.matmul(
                            ps["v"][:],
                            wv_sbs[wi][:, cbl * 128 : (cbl + 1) * 128],
                            xtr, start=st, stop=sp,
                        )
                    # RoPE in pair-interleaved head layout (host permuted Wq/Wk
                    # columns so rotate-half pairs are adjacent partitions):
                    # dest = psum*cosI + swap_adjacent(psum)*sinS
                    swap_mask = [i ^ 1 for i in range(32)]
                    for kind, dest in (("q0", QT[0]), ("q1", QT[1]), ("k", KT)):
                        ra = wkp.tile([128, 512], f32, tag="ra", bufs=3)
                        rb = wkp.tile([128, 512], f32, tag="rb", bufs=3)
                        nc.vector.tensor_mul(ra[:], ps[kind][:], cos_sb[:, ts_])
                        nc.vector.stream_shuffle(rb[:], ps[kind][:], swap_mask)
                        nc.vector.tensor_mul(rb[:], rb[:], sin_sb[:, ts_])
                        nc.vector.tensor_add(dest[:, ts_], ra[:], rb[:])
                    nc.vector.tensor_copy(VT[:, ts_], ps["v"][:])

                # ---- V natural ([t,d] blocks) via PE transpose ----
                for j in range(t // 128):
                    pt = pp.tile([128, 128], f32, tag="av", bufs=1)
                    nc.tensor.transpose(pt[:], VT[:, j * 128 : (j + 1) * 128], id_sb[:])
                    nc.vector.tensor_copy(Vn[:, j * 128 : (j + 1) * 128], pt[:])

                # ---- attention (+ interleaved output projection) ----
                for i4 in range(nt):
                    for h in range(HL):
                        qs = slice(i4 * 512, (i4 + 1) * 512)
                        pav = pp.tile([128, 512], f32, tag="av", bufs=1)
                        pden = pp.tile([1, 512], f32, tag="den", bufs=1)
                        jmax = njb_per_t * (i4 + 1) - 1
                        for j in range(jmax + 1):
                            diag = j - njb_per_t * i4
                            off = max(diag, 0) * 128  # skip q cols left of diag
                            pst = pp.tile([128, 512], f32, tag="s", bufs=2)
                            nc.tensor.matmul(
                                pst[:, off:512],
                                KT[:, j * 128 : (j + 1) * 128],
                                QT[h][:, i4 * 512 + off : (i4 + 1) * 512],
                                start=True, stop=True,
                            )
                            E = wkp.tile([128, 512], f32r, tag="E", bufs=6)
                            nc.scalar.activation(
                                E[:, off:512], pst[:, off:512], EXP, scale=F32R_SCALE
                            )
                            if diag >= 0:
                                # zero strictly-lower triangle of the diag block
                                nc.vector.tensor_mul(
                                    E[:, off : off + 128],
                                    E[:, off : off + 128],
                                    mask_sb[:, 384:512],
                                )
                            nc.tensor.matmul(
                                pav[:, off:512],
                                Vn[:, j * 128 : (j + 1) * 128],
                                E[:, off:512],
                                start=(j == 0), stop=(j == jmax),
                                skip_group_check=True,
                            )
                            nc.tensor.matmul(
                                pden[:, off:512],
                                ones_sb[:, 0:1],
                                E[:, off:512],
                                start=(j == 0), stop=(j == jmax),
                                skip_group_check=True,
                            )
                        rec = wkp.tile([1, 512], f32, tag="rec", bufs=2)
                        nc.vector.reciprocal_approx_fast(rec[:], pden[:])
                        rbc = wkp.tile([128, 512], f32, tag="rbc", bufs=2)
                        nc.gpsimd.partition_broadcast(rbc[:], rec[:])
                        nc.vector.tensor_mul(AT[h][:, qs], pav[:], rbc[:])

                    # ---- output projection for the rows this i4 finished ----
                    for it in range(i4 * 4, (i4 + 1) * 4):
                        for n in range(c // 512):
                            po = pp.tile([128, 512], f32, tag="proj", bufs=4)
                            for hh in range(HL):
                                nc.tensor.matmul(
                                    po[:],
                                    AT[hh][:, it * 128 : (it + 1) * 128],
                                    wp_sb[:, hh * c + n * 512 : hh * c + (n + 1) * 512],
                                    start=(hh == 0), stop=(hh == HL - 1),
                                )
                            po_sb = wkp.tile([128, 512], f32, tag="yout", bufs=6)
                            if (it * (c // 512) + n) % 2 == 0:
                                nc.vector.tensor_copy(po_sb[:], po[:])
                            else:
                                nc.scalar.copy(po_sb[:], po[:])
                            nc.sync.dma_start(
                                y[bi, it * 128 : (it + 1) * 128, n * 512 : (n + 1) * 512],
                                po_sb[:],
                            )

    nc.compile()
    return nc


def host_inputs(x, Wq, Wk, Wv, Wp, ncores=NCORES, mmdt="f32r"):
    import ml_dtypes

    mdt = np.float32 if mmdt == "f32r" else ml_dtypes.bfloat16
    """Per-core input dicts (sharding + layout prep on host)."""
    b, t, c = x.shape
    d = D
    xT = np.ascontiguousarray(np.transpose(x, (0, 2, 1)))  # [B, C, T]
    inv = (1.0 / (10000.0 ** (np.arange(0, d, 2, dtype=np.float32) / np.float32(d)))).astype(np.float32)
    pos = np.arange(t, dtype=np.float32)
    fr = np.outer(pos, inv).astype(np.float32)  # [T, 64]
    cosT = np.cos(fr).T.astype(np.float32)  # [64, T]
    sinT = np.sin(fr).T.astype(np.float32)
    # pair-interleaved rope tables: partition 2m,2m+1 <- freq m; sign -/+ on sin
    cosI = np.ascontiguousarray(np.repeat(cosT, 2, axis=0))  # [128, T]
    sinS = np.ascontiguousarray(
        np.stack([-sinT, sinT], axis=1).reshape(128, t)
    )
    # column permutation putting rope pair (m, m+64) at (2m, 2m+1), per head
    perm = np.stack([np.arange(64), np.arange(64) + 64], 1).reshape(128)
    triu = np.triu(np.ones((128, 128), np.float32))
    maskf = np.ascontiguousarray(
        np.concatenate([np.zeros((128, 384), np.float32), triu], 1)
    )
    onesv = np.ones((128, 1), np.float32)
    ident = np.eye(128, dtype=np.float32)

    def permute_heads(w):
        # w: [c, nheads*d] -> same with each head's columns permuted by perm
        nh = w.shape[1] // d
        wv_ = w.reshape(w.shape[0], nh, d)
        return np.ascontiguousarray(wv_[:, :, perm].reshape(w.shape))

    Wq_p = permute_heads(Wq)
    Wk_p = permute_heads(Wk)

    xTm = xT.astype(mdt) if mdt is not np.float32 else xT
    in_maps = []
    for ci in range(ncores):
        qs = slice(ci * HL * d, (ci + 1) * HL * d)
        in_maps.append(
            {
                "xT": xTm,
                "wq": np.ascontiguousarray(Wq_p[:, qs]).astype(mdt),
                "wk": np.ascontiguousarray(Wk_p[:, ci * d : (ci + 1) * d]).astype(mdt),
                "wv": np.ascontiguousarray(Wv[:, ci * d : (ci + 1) * d]).astype(mdt),
                "wp": np.ascontiguousarray(Wp[qs, :]).astype(mdt),
                "cos2": cosI,
                "sin2": sinS,
                "maskf": maskf.astype(mdt),
                "onesv": onesv.astype(mdt),
                "ident": ident,
            }
        )
    return in_maps


_NC_CACHE = {}

MMDT = "f32r"


def _get_nc(mmdt=None):
    mmdt = mmdt or MMDT
    key = (B, T, C, mmdt)
    if key not in _NC_CACHE:
        _NC_CACHE[key] = build_nc(B, T, C, mmdt=mmdt)
    return _NC_CACHE[key]


def _install_cc_error_surfacing():
    """Make neuronx_cc hook failures print a real traceback instead of the
    opaque PJRT 'py_result' error."""
    try:
        from concourse import bass2jax

        bass2jax.install_neuronx_cc_hook()
        import libneuronxla

        if getattr(libneuronxla, "_tb_wrapped", False):
            return
        inner = libneuronxla.neuronx_cc

        def wrapped(*a, **k):
            try:
                return inner(*a, **k)
            except BaseException:
                import traceback

                traceback.print_exc()
                raise

        libneuronxla.neuronx_cc = wrapped
        libneuronxla._tb_wrapped = True
    except Exception:
        pass


def run_spmd(x, Wq, Wk, Wv, Wp, trace=False, mmdt=None):
    from concourse.bass_utils import run_bass_kernel_spmd

    mmdt = mmdt or MMDT
    _install_cc_error_surfacing()

    nc = _get_nc(mmdt)
    in_maps = host_inputs(x, Wq, Wk, Wv, Wp, mmdt=mmdt)
    last_err = None
    for attempt in range(3):
        try:
            res = run_bass_kernel_spmd(
                nc, in_maps, core_ids=list(range(NCORES)), trace=trace
            )
            break
        except Exception as e:  # transient NRT device faults: retry
            last_err = e
            import time as _time

            _time.sleep(5.0)
    else:
        raise last_err
    acc = res.results[0]["y"].astype(np.float64)
    for i in range(1, NCORES):
        acc += res.results[i]["y"]
    return acc.astype(np.float32), res


def kernel(x, Wq, Wk, Wv, Wp):
    out, _ = run_spmd(x, Wq, Wk, Wv, Wp, trace=False)
    return out
